# revision 1
# baseline (speedup 1.0000x reference)
"""Trainium2 kernel for nn_MeshDeformationBlock (5-layer GATv2 stack) on 8 cores."""
import sys

sys.path.insert(0, "/root/problem")
import numpy as np

import kernel_lib as kl

_cache = {}


def _prepare_and_build(x, params, edge_index, trace=False):
    x = np.asarray(x, np.float32)
    params = [{k: np.asarray(v) for k, v in p.items()} for p in params]
    ei = np.asarray(edge_index)
    part, layers, cfgs, xt0_per_core, wcats, ident = kl.host_prepare(x, params, ei)
    deg_slot = [int(d) for d in part["deg_slot"]]
    key = (tuple(deg_slot), tuple((c.fo, c.n_pos) for c in cfgs))
    if key not in _cache:
        _cache[key] = kl.build_program(deg_slot, cfgs, [0.0] * 5, dbg=False)
    nc = _cache[key]
    in_maps = []
    for c in range(kl.N_CORES):
        m = {"xt0": xt0_per_core[c], "gidx": part["idx_per_core"][c], "ident": ident}
        for li in range(5):
            m[f"wcat{li}"] = wcats[li]
        in_maps.append(m)
    return nc, in_maps, part, layers


def kernel(x, params, edge_index):
    nc, in_maps, part, layers = _prepare_and_build(x, params, edge_index)
    res = kl.run_bass_kernel_spmd(nc, in_maps, core_ids=list(range(kl.N_CORES)))
    return kl.assemble_outputs(res.results, part, layers)


def kernel_traced(x, params, edge_index):
    nc, in_maps, part, layers = _prepare_and_build(x, params, edge_index)
    res = kl.run_bass_kernel_spmd(
        nc, in_maps, core_ids=list(range(kl.N_CORES)), trace=True
    )
    out, coords = kl.assemble_outputs(res.results, part, layers)
    return out, coords, res
